# revision 21
# baseline (speedup 1.0000x reference)
"""Bahdanau additive-attention kernel for Trainium2, 8 NeuronCores. v3.

Problem (B=32, S=2048, H=1024, E=2H):
    hid_proj = hidden @ w_h.T + attn_b                  # (B, H)   host
    enc_proj[b,s,h] = sum_e enc[b,s,e] * w_e[h,e]       # (B, S, H) PE (dominant)
    energy = tanh(hid_proj[:,None,:] + enc_proj)        # ACT
    scores[b,s] = sum_h energy[b,s,h] * v_w[h]          # PE (v-dot)
    attw = softmax(scores, axis=1)                      # exp on ACT, /Z on host
    context[b,e] = sum_s attw[b,s] * enc[b,s,e]         # DVE (mult+reduce)

Sharding: data-parallel over batch, 4 batches per core.

Design notes (vs the 527us v1 baseline):
  - enc pre-transposed ON HOST to [b, e, s] bf16: plain contiguous DMAs
    instead of 8.3us xbar DMA-transposes; one resident tile serves both
    the main matmul (rhs) and the context reduction (in0). Halves HBM.
  - context off the PE (was 55us rank-1 matmuls + 64 transposes): exp row
    replicated across partitions via K=128 ones-matmul on a zero-padded
    row tile (K=1 replication returns zeros on HW), then per (st, k)
    DVE tensor_tensor mult + tensor_reduce partials, accumulated with a
    final tiny reduce. Per-st pipelining keeps all but the last s-tile's
    DVE work hidden under the next main matmuls. (Fused
    tensor_tensor_reduce hangs the device - do not use.)
  - softmax 1/Z on host: kernel outputs raw bf16 exp row, Z partials,
    unnormalized ctx columns.
  - startup: 16 PE warmup matmuls (HAM un-throttles after ~3.4us of
    activity); weights loaded m-major on the Scalar queue while batch 0's
    encT arrives in (k, st)-pieces on Sync/GpSimd, so the first m-loop
    starts at ~4us with no DMA stall (a stall >3.4us re-throttles the PE
    clock to 1.2GHz - costs ~45us).
  - MM issue rate is 216ns at N=512 with per-MM LDWEIGHTS (background
    weight-buffer load is free); no stationary-reuse tricks needed.
"""

import numpy as np
import ml_dtypes

import concourse.bass as bass
import concourse.tile as tile
import concourse.mybir as mybir
from concourse import bacc
from concourse.bass_utils import run_bass_kernel_spmd

B, S, H = 32, 2048, 1024
E = 2 * H
NCORES = 8
BL = B // NCORES          # batches per core
P = 128                   # partitions
KC = E // P               # 16 contraction chunks
MC = H // P               # 8 h chunks
NT = 512                  # moving free-dim per matmul (1 PSUM bank of fp32)
ST = S // NT              # 4 s-tiles per batch

F32 = mybir.dt.float32
BF16 = mybir.dt.bfloat16
BF16_NP = ml_dtypes.bfloat16


def build_nc():
    nc = bacc.Bacc("TRN2", target_bir_lowering=False, debug=False)

    # encT[b, k, p, s] = enc[b, s, k*128+p], pre-transposed + bf16 on host
    enc_in = nc.dram_tensor("encT", [BL, KC, P, S], BF16, kind="ExternalInput")
    # wT[k, p, h] = w_e[h, k*128+p]. Keep w_sb k-major [P, KC, H]: an
    # m-major layout measures 250ns/MM instead of 216 (SBUF read-path
    # conflict between the LDWEIGHTS and rhs streams).
    w_in = nc.dram_tensor("wT", [KC, P, H], BF16, kind="ExternalInput")
    v_in = nc.dram_tensor("vcol", [P, MC], BF16, kind="ExternalInput")
    hid_in = nc.dram_tensor("hidc", [P, BL, MC], F32, kind="ExternalInput")
    ctx_out = nc.dram_tensor("ctxc", [BL, P, KC], F32, kind="ExternalOutput")
    exp_out = nc.dram_tensor("expr", [BL, S], BF16, kind="ExternalOutput")
    z_out = nc.dram_tensor("zp", [BL, ST], F32, kind="ExternalOutput")

    with tile.TileContext(nc) as tc:
        with (
            tc.tile_pool(name="singles", bufs=1) as singles,
            tc.tile_pool(name="pencT", bufs=2) as pencT,
            tc.tile_pool(name="pen", bufs=2) as pen,
            tc.tile_pool(name="prow", bufs=1) as prow,
            tc.tile_pool(name="pmm", bufs=3, space="PSUM") as pmm,
            tc.tile_pool(name="psc", bufs=2, space="PSUM") as psc,
            tc.tile_pool(name="prep", bufs=2, space="PSUM") as prep,
        ):
            # --- warmup operands (memset only - ready in ~1us) -----------
            warm = singles.tile([P, P], BF16)
            nc.vector.memset(warm, 0.5)
            exrow_pad = singles.tile([P, S], BF16)
            nc.vector.memset(exrow_pad[:, 0:NT], 0.0)
            nc.vector.memset(exrow_pad[:, NT:S], 0.0)
            ones128 = singles.tile([P, P], BF16)
            nc.vector.memset(ones128, 1.0)

            # --- PE warmup: junk matmuls (~4.3us) to lift HAM to 2.4GHz
            for wi in range(20):
                wp = prep.tile([P, NT], F32, name=f"warm{wi}", tag="rep")
                nc.tensor.matmul(wp, lhsT=warm, rhs=exrow_pad[:, 0:NT],
                                 start=True, stop=True)

            # Scalar queue stays free for ACT (a busy Scalar queue delays
            # the first tanh and stalls the PE on PSUM-pool rotation)
            w_sb = singles.tile([P, KC, H], BF16)
            v_sb = singles.tile([P, MC], BF16)
            nc.gpsimd.dma_start(out=v_sb, in_=v_in[:, :])
            hid_sb = singles.tile([P, BL, MC], F32)
            nc.gpsimd.dma_start(out=hid_sb, in_=hid_in[:, :, :])

            exrep = singles.tile([P, S], BF16)
            scratch = singles.tile([P, NT], BF16)
            scratch_g = singles.tile([P, NT], BF16)
            scratch_g2 = singles.tile([P, NT], BF16)
            scratch_o = singles.tile([P, NT], BF16)

            # --- startup streaming, paced to the first m-loop ------------
            # st0 pieces go on the Scalar queue (it is idle until the first
            # tanh at ~15us), w per-k chunks interleave across Sync/GpSimd:
            # m0's k-loop starts at ~5us and every chunk lands just ahead
            # of its MM.
            encT = {}
            encT[0] = pencT.tile([P, KC, S], BF16, name="encT0", tag="encT")
            for k in range(KC):
                nc.scalar.dma_start(out=encT[0][:, k, 0:NT],
                                    in_=enc_in[0, k, :, 0:NT])
                q = nc.gpsimd if k % 2 else nc.sync
                q.dma_start(out=w_sb[:, k, :], in_=w_in[k])
            for st in range(1, ST):
                for k in range(KC):
                    q = nc.gpsimd if k % 2 else nc.sync
                    q.dma_start(out=encT[0][:, k, st * NT:(st + 1) * NT],
                                in_=enc_in[0, k, :, st * NT:(st + 1) * NT])
            for b in range(1, BL):
                encT[b] = pencT.tile([P, KC, S], BF16, name=f"encT{b}", tag="encT")
                nc.sync.dma_start(
                    out=encT[b][:, 0:KC // 2, :],
                    in_=enc_in[b, 0:KC // 2].rearrange("k p s -> p k s"))
                nc.gpsimd.dma_start(
                    out=encT[b][:, KC // 2:KC, :],
                    in_=enc_in[b, KC // 2:KC].rearrange("k p s -> p k s"))

            for b in range(BL):
                parts = prow.tile([P, KC, ST], F32, name=f"parts{b}", tag="parts")
                for st in range(ST):
                    # ---- main: enc_proj + tanh, one m-chunk at a time ---
                    en = pen.tile([P, MC, NT], BF16, name=f"en{b}_{st}", tag="en")
                    for m in range(MC):
                        ps = pmm.tile([P, NT], F32, name=f"ps{b}_{st}_{m}", tag="mm")
                        for k in range(KC):
                            nc.tensor.matmul(
                                ps,
                                lhsT=w_sb[:, k, m * P:(m + 1) * P],
                                rhs=encT[b][:, k, st * NT:(st + 1) * NT],
                                start=(k == 0),
                                stop=(k == KC - 1),
                            )
                        nc.scalar.activation(
                            out=en[:, m, :],
                            in_=ps,
                            func=mybir.ActivationFunctionType.Tanh,
                            bias=hid_sb[:, b, m:m + 1],
                            scale=1.0,
                        )
                    # ---- scores: v-dot on PE ----------------------------
                    sc = psc.tile([1, NT], F32, name=f"sc{b}_{st}", tag="sc")
                    for m in range(MC):
                        nc.tensor.matmul(
                            sc,
                            lhsT=v_sb[:, m:m + 1],
                            rhs=en[:, m, :],
                            start=(m == 0),
                            stop=(m == MC - 1),
                        )
                    # ---- exp straight from PSUM into padded row 0 -------
                    zt = prow.tile([1, 1], F32, name=f"z{b}_{st}", tag=f"z{st}")
                    nc.scalar.activation(
                        out=exrow_pad[0:1, st * NT:(st + 1) * NT],
                        in_=sc,
                        func=mybir.ActivationFunctionType.Exp,
                        accum_out=zt,
                    )
                    nc.scalar.dma_start(out=z_out[b, st:st + 1], in_=zt)

                    # ---- replicate exp slice across partitions ----------
                    rp = prep.tile([P, NT], F32, name=f"rp{b}_{st}", tag="rep")
                    nc.tensor.matmul(rp, lhsT=ones128,
                                     rhs=exrow_pad[:, st * NT:(st + 1) * NT],
                                     start=True, stop=True)
                    nc.scalar.activation(
                        out=exrep[:, st * NT:(st + 1) * NT],
                        in_=rp,
                        func=mybir.ActivationFunctionType.Copy,
                    )

                    # ---- context partials on DVE ------------------------
                    # the last (b, st) is on the critical path (nothing
                    # left to hide it under): split across GpSimd-mult +
                    # ACT-accum-reduce alongside the DVE chain
                    last = (b == BL - 1 and st == ST - 1)
                    for k in range(KC):
                        if last and k >= 10:
                            sg = scratch_g if k % 2 else scratch_g2
                            nc.gpsimd.tensor_tensor(
                                out=sg,
                                in0=encT[b][:, k, st * NT:(st + 1) * NT],
                                in1=exrep[:, st * NT:(st + 1) * NT],
                                op=mybir.AluOpType.mult)
                            nc.scalar.activation(
                                out=scratch_o, in_=sg,
                                func=mybir.ActivationFunctionType.Copy,
                                accum_out=parts[:, k, st:st + 1])
                        else:
                            nc.vector.tensor_tensor(
                                out=scratch,
                                in0=encT[b][:, k, st * NT:(st + 1) * NT],
                                in1=exrep[:, st * NT:(st + 1) * NT],
                                op=mybir.AluOpType.mult)
                            nc.vector.tensor_reduce(
                                out=parts[:, k, st:st + 1], in_=scratch,
                                axis=mybir.AxisListType.XYZW,
                                op=mybir.AluOpType.add)

                # ---- batch tail: raw outputs ----------------------------
                nc.scalar.dma_start(out=exp_out[b], in_=exrow_pad[0:1, :])
                ctxcols = prow.tile([P, KC], F32, name=f"cc{b}", tag="cc")
                nc.vector.tensor_reduce(out=ctxcols, in_=parts,
                                        axis=mybir.AxisListType.X,
                                        op=mybir.AluOpType.add)
                nc.gpsimd.dma_start(out=ctx_out[b], in_=ctxcols)

    nc.compile()
    return nc


_CACHE = {}


def _get_nc():
    if "nc" not in _CACHE:
        _CACHE["nc"] = build_nc()
    return _CACHE["nc"]


def prep_in_maps(hidden, encoder_outputs, attn_w, attn_b, v_w):
    hidden = np.asarray(hidden, dtype=np.float32)
    enc = np.asarray(encoder_outputs, dtype=np.float32)
    attn_w = np.asarray(attn_w, dtype=np.float32)
    attn_b = np.asarray(attn_b, dtype=np.float32)
    v_w = np.asarray(v_w, dtype=np.float32)

    w_h = attn_w[:, :H]                       # (H, H)
    w_e = attn_w[:, H:]                       # (H, E)
    hid_proj = hidden @ w_h.T + attn_b        # (B, H) fp32, exact
    wT = np.ascontiguousarray(w_e.T).astype(BF16_NP).reshape(KC, P, H)
    vcol = np.ascontiguousarray(v_w.reshape(MC, P).T).astype(BF16_NP)  # (P, MC)

    # pre-transpose enc to [b, e, s] bf16, viewed as [b, KC, P, S]
    encT = np.ascontiguousarray(
        enc.transpose(0, 2, 1).astype(BF16_NP)
    ).reshape(B, KC, P, S)

    in_maps = []
    for c in range(NCORES):
        hp = hid_proj[c * BL:(c + 1) * BL]    # (BL, H)
        hidc = np.ascontiguousarray(hp.reshape(BL, MC, P).transpose(2, 0, 1))
        in_maps.append({
            "encT": encT[c * BL:(c + 1) * BL],
            "wT": wT,
            "vcol": vcol,
            "hidc": hidc.astype(np.float32),
        })
    return in_maps


def kernel(hidden, encoder_outputs, attn_w, attn_b, v_w):
    in_maps = prep_in_maps(hidden, encoder_outputs, attn_w, attn_b, v_w)
    nc = _get_nc()
    res = run_bass_kernel_spmd(nc, in_maps, core_ids=list(range(NCORES)))

    ctx = np.empty((B, E), dtype=np.float32)
    attw = np.empty((B, S), dtype=np.float32)
    for c in range(NCORES):
        r = res.results[c]
        exp_rows = r["expr"].astype(np.float32)          # (BL, S)
        z = r["zp"].astype(np.float32).sum(axis=1)       # (BL,)
        ctxc = r["ctxc"].astype(np.float32)              # (BL, P, KC)
        for b in range(BL):
            gb = c * BL + b
            attw[gb] = exp_rows[b] / z[b]
            # ctxc[b, p, k] -> ctx[e] with e = k*128 + p
            ctx[gb] = ctxc[b].T.reshape(E) / z[b]
    return ctx, attw


# revision 23
# speedup vs baseline: 1.0011x; 1.0011x over previous
"""Bahdanau additive-attention kernel for Trainium2, 8 NeuronCores. v3.

Problem (B=32, S=2048, H=1024, E=2H):
    hid_proj = hidden @ w_h.T + attn_b                  # (B, H)   host
    enc_proj[b,s,h] = sum_e enc[b,s,e] * w_e[h,e]       # (B, S, H) PE (dominant)
    energy = tanh(hid_proj[:,None,:] + enc_proj)        # ACT
    scores[b,s] = sum_h energy[b,s,h] * v_w[h]          # PE (v-dot)
    attw = softmax(scores, axis=1)                      # exp on ACT, /Z on host
    context[b,e] = sum_s attw[b,s] * enc[b,s,e]         # DVE (mult+reduce)

Sharding: data-parallel over batch, 4 batches per core.

Design notes (vs the 527us v1 baseline):
  - enc pre-transposed ON HOST to [b, e, s] bf16: plain contiguous DMAs
    instead of 8.3us xbar DMA-transposes; one resident tile serves both
    the main matmul (rhs) and the context reduction (in0). Halves HBM.
  - context off the PE (was 55us rank-1 matmuls + 64 transposes): exp row
    replicated across partitions via K=128 ones-matmul on a zero-padded
    row tile (K=1 replication returns zeros on HW), then per (st, k)
    DVE tensor_tensor mult + tensor_reduce partials, accumulated with a
    final tiny reduce. Per-st pipelining keeps all but the last s-tile's
    DVE work hidden under the next main matmuls. (Fused
    tensor_tensor_reduce hangs the device - do not use.)
  - softmax 1/Z on host: kernel outputs raw bf16 exp row, Z partials,
    unnormalized ctx columns.
  - startup: 16 PE warmup matmuls (HAM un-throttles after ~3.4us of
    activity); weights loaded m-major on the Scalar queue while batch 0's
    encT arrives in (k, st)-pieces on Sync/GpSimd, so the first m-loop
    starts at ~4us with no DMA stall (a stall >3.4us re-throttles the PE
    clock to 1.2GHz - costs ~45us).
  - MM issue rate is 216ns at N=512 with per-MM LDWEIGHTS (background
    weight-buffer load is free); no stationary-reuse tricks needed.
"""

import numpy as np
import ml_dtypes

import concourse.bass as bass
import concourse.tile as tile
import concourse.mybir as mybir
from concourse import bacc
from concourse.bass_utils import run_bass_kernel_spmd

B, S, H = 32, 2048, 1024
E = 2 * H
NCORES = 8
BL = B // NCORES          # batches per core
P = 128                   # partitions
KC = E // P               # 16 contraction chunks
MC = H // P               # 8 h chunks
NT = 512                  # moving free-dim per matmul (1 PSUM bank of fp32)
ST = S // NT              # 4 s-tiles per batch

F32 = mybir.dt.float32
BF16 = mybir.dt.bfloat16
BF16_NP = ml_dtypes.bfloat16


def build_nc():
    nc = bacc.Bacc("TRN2", target_bir_lowering=False, debug=False)

    # encT[b, k, p, s] = enc[b, s, k*128+p], pre-transposed + bf16 on host
    enc_in = nc.dram_tensor("encT", [BL, KC, P, S], BF16, kind="ExternalInput")
    # wT[k, p, h] = w_e[h, k*128+p]. Keep w_sb k-major [P, KC, H]: an
    # m-major layout measures 250ns/MM instead of 216 (SBUF read-path
    # conflict between the LDWEIGHTS and rhs streams).
    w_in = nc.dram_tensor("wT", [KC, P, H], BF16, kind="ExternalInput")
    v_in = nc.dram_tensor("vcol", [P, MC], BF16, kind="ExternalInput")
    hid_in = nc.dram_tensor("hidc", [P, BL, MC], F32, kind="ExternalInput")
    ctx_out = nc.dram_tensor("ctxc", [BL, P, KC], F32, kind="ExternalOutput")
    exp_out = nc.dram_tensor("expr", [BL, S], BF16, kind="ExternalOutput")
    z_out = nc.dram_tensor("zp", [BL, ST], F32, kind="ExternalOutput")

    with tile.TileContext(nc) as tc:
        with (
            tc.tile_pool(name="singles", bufs=1) as singles,
            tc.tile_pool(name="pencT", bufs=2) as pencT,
            tc.tile_pool(name="pen", bufs=2) as pen,
            tc.tile_pool(name="prow", bufs=1) as prow,
            tc.tile_pool(name="pmm", bufs=3, space="PSUM") as pmm,
            tc.tile_pool(name="psc", bufs=2, space="PSUM") as psc,
            tc.tile_pool(name="prep", bufs=2, space="PSUM") as prep,
        ):
            # --- warmup operands (memset only - ready in ~1us) -----------
            warm = singles.tile([P, P], BF16)
            nc.vector.memset(warm, 0.5)
            exrow_pad = singles.tile([P, S], BF16)
            nc.vector.memset(exrow_pad[:, 0:NT], 0.0)
            nc.vector.memset(exrow_pad[:, NT:S], 0.0)
            ones128 = singles.tile([P, P], BF16)
            nc.vector.memset(ones128, 1.0)

            # --- PE warmup: junk matmuls (~4.3us) to lift HAM to 2.4GHz
            for wi in range(20):
                wp = prep.tile([P, NT], F32, name=f"warm{wi}", tag="rep")
                nc.tensor.matmul(wp, lhsT=warm, rhs=exrow_pad[:, 0:NT],
                                 start=True, stop=True)

            # Scalar queue stays free for ACT (a busy Scalar queue delays
            # the first tanh and stalls the PE on PSUM-pool rotation)
            w_sb = singles.tile([P, KC, H], BF16)
            v_sb = singles.tile([P, MC], BF16)
            nc.gpsimd.dma_start(out=v_sb, in_=v_in[:, :])
            hid_sb = singles.tile([P, BL, MC], F32)
            nc.gpsimd.dma_start(out=hid_sb, in_=hid_in[:, :, :])

            exrep = singles.tile([P, S], BF16)
            scratch = singles.tile([P, NT], BF16)

            # --- startup streaming, paced to the first m-loop ------------
            # st0 pieces go on the Scalar queue (it is idle until the first
            # tanh at ~15us), w per-k chunks interleave across Sync/GpSimd:
            # m0's k-loop starts at ~5us and every chunk lands just ahead
            # of its MM.
            encT = {}
            encT[0] = pencT.tile([P, KC, S], BF16, name="encT0", tag="encT")
            for k in range(KC):
                nc.scalar.dma_start(out=encT[0][:, k, 0:NT],
                                    in_=enc_in[0, k, :, 0:NT])
                q = nc.gpsimd if k % 2 else nc.sync
                q.dma_start(out=w_sb[:, k, :], in_=w_in[k])
            for st in range(1, ST):
                for k in range(KC):
                    q = nc.gpsimd if k % 2 else nc.sync
                    q.dma_start(out=encT[0][:, k, st * NT:(st + 1) * NT],
                                in_=enc_in[0, k, :, st * NT:(st + 1) * NT])
            for b in range(1, BL):
                encT[b] = pencT.tile([P, KC, S], BF16, name=f"encT{b}", tag="encT")
                nc.sync.dma_start(
                    out=encT[b][:, 0:KC // 2, :],
                    in_=enc_in[b, 0:KC // 2].rearrange("k p s -> p k s"))
                nc.gpsimd.dma_start(
                    out=encT[b][:, KC // 2:KC, :],
                    in_=enc_in[b, KC // 2:KC].rearrange("k p s -> p k s"))

            # NOTE: these live in prow (the LAST pool) on purpose - adding
            # them to `singles` shifts the encT pool base by 3KB, which
            # changes the SBUF bank phase between the LDWEIGHTS and rhs
            # read streams and degrades the MM issue rate 216 -> 250ns.
            scratch_g = prow.tile([P, NT], BF16, tag="sg")
            scratch_g2 = prow.tile([P, NT], BF16, tag="sg2")
            scratch_o = prow.tile([P, NT], BF16, tag="so")

            for b in range(BL):
                parts = prow.tile([P, KC, ST], F32, name=f"parts{b}", tag="parts")
                for st in range(ST):
                    # ---- main: enc_proj + tanh, one m-chunk at a time ---
                    en = pen.tile([P, MC, NT], BF16, name=f"en{b}_{st}", tag="en")
                    for m in range(MC):
                        ps = pmm.tile([P, NT], F32, name=f"ps{b}_{st}_{m}", tag="mm")
                        for k in range(KC):
                            nc.tensor.matmul(
                                ps,
                                lhsT=w_sb[:, k, m * P:(m + 1) * P],
                                rhs=encT[b][:, k, st * NT:(st + 1) * NT],
                                start=(k == 0),
                                stop=(k == KC - 1),
                            )
                        nc.scalar.activation(
                            out=en[:, m, :],
                            in_=ps,
                            func=mybir.ActivationFunctionType.Tanh,
                            bias=hid_sb[:, b, m:m + 1],
                            scale=1.0,
                        )
                    # ---- scores: v-dot on PE ----------------------------
                    sc = psc.tile([1, NT], F32, name=f"sc{b}_{st}", tag="sc")
                    for m in range(MC):
                        nc.tensor.matmul(
                            sc,
                            lhsT=v_sb[:, m:m + 1],
                            rhs=en[:, m, :],
                            start=(m == 0),
                            stop=(m == MC - 1),
                        )
                    # ---- exp straight from PSUM into padded row 0 -------
                    zt = prow.tile([1, 1], F32, name=f"z{b}_{st}", tag=f"z{st}")
                    nc.scalar.activation(
                        out=exrow_pad[0:1, st * NT:(st + 1) * NT],
                        in_=sc,
                        func=mybir.ActivationFunctionType.Exp,
                        accum_out=zt,
                    )
                    nc.scalar.dma_start(out=z_out[b, st:st + 1], in_=zt)

                    # ---- replicate exp slice across partitions ----------
                    rp = prep.tile([P, NT], F32, name=f"rp{b}_{st}", tag="rep")
                    nc.tensor.matmul(rp, lhsT=ones128,
                                     rhs=exrow_pad[:, st * NT:(st + 1) * NT],
                                     start=True, stop=True)
                    nc.scalar.activation(
                        out=exrep[:, st * NT:(st + 1) * NT],
                        in_=rp,
                        func=mybir.ActivationFunctionType.Copy,
                    )

                    # ---- context partials on DVE ------------------------
                    # the last (b, st) is on the critical path (nothing
                    # left to hide it under): split across GpSimd-mult +
                    # ACT-accum-reduce alongside the DVE chain
                    last = (b == BL - 1 and st == ST - 1)
                    for k in range(KC):
                        if last and k >= 10:
                            sg = scratch_g if k % 2 else scratch_g2
                            nc.gpsimd.tensor_tensor(
                                out=sg,
                                in0=encT[b][:, k, st * NT:(st + 1) * NT],
                                in1=exrep[:, st * NT:(st + 1) * NT],
                                op=mybir.AluOpType.mult)
                            nc.scalar.activation(
                                out=scratch_o, in_=sg,
                                func=mybir.ActivationFunctionType.Copy,
                                accum_out=parts[:, k, st:st + 1])
                        else:
                            nc.vector.tensor_tensor(
                                out=scratch,
                                in0=encT[b][:, k, st * NT:(st + 1) * NT],
                                in1=exrep[:, st * NT:(st + 1) * NT],
                                op=mybir.AluOpType.mult)
                            nc.vector.tensor_reduce(
                                out=parts[:, k, st:st + 1], in_=scratch,
                                axis=mybir.AxisListType.XYZW,
                                op=mybir.AluOpType.add)

                # ---- batch tail: raw outputs ----------------------------
                nc.scalar.dma_start(out=exp_out[b], in_=exrow_pad[0:1, :])
                ctxcols = prow.tile([P, KC], F32, name=f"cc{b}", tag="cc")
                nc.vector.tensor_reduce(out=ctxcols, in_=parts,
                                        axis=mybir.AxisListType.X,
                                        op=mybir.AluOpType.add)
                nc.gpsimd.dma_start(out=ctx_out[b], in_=ctxcols)

    nc.compile()
    return nc


_CACHE = {}


def _get_nc():
    if "nc" not in _CACHE:
        _CACHE["nc"] = build_nc()
    return _CACHE["nc"]


def prep_in_maps(hidden, encoder_outputs, attn_w, attn_b, v_w):
    hidden = np.asarray(hidden, dtype=np.float32)
    enc = np.asarray(encoder_outputs, dtype=np.float32)
    attn_w = np.asarray(attn_w, dtype=np.float32)
    attn_b = np.asarray(attn_b, dtype=np.float32)
    v_w = np.asarray(v_w, dtype=np.float32)

    w_h = attn_w[:, :H]                       # (H, H)
    w_e = attn_w[:, H:]                       # (H, E)
    hid_proj = hidden @ w_h.T + attn_b        # (B, H) fp32, exact
    wT = np.ascontiguousarray(w_e.T).astype(BF16_NP).reshape(KC, P, H)
    vcol = np.ascontiguousarray(v_w.reshape(MC, P).T).astype(BF16_NP)  # (P, MC)

    # pre-transpose enc to [b, e, s] bf16, viewed as [b, KC, P, S]
    encT = np.ascontiguousarray(
        enc.transpose(0, 2, 1).astype(BF16_NP)
    ).reshape(B, KC, P, S)

    in_maps = []
    for c in range(NCORES):
        hp = hid_proj[c * BL:(c + 1) * BL]    # (BL, H)
        hidc = np.ascontiguousarray(hp.reshape(BL, MC, P).transpose(2, 0, 1))
        in_maps.append({
            "encT": encT[c * BL:(c + 1) * BL],
            "wT": wT,
            "vcol": vcol,
            "hidc": hidc.astype(np.float32),
        })
    return in_maps


def kernel(hidden, encoder_outputs, attn_w, attn_b, v_w):
    in_maps = prep_in_maps(hidden, encoder_outputs, attn_w, attn_b, v_w)
    nc = _get_nc()
    res = run_bass_kernel_spmd(nc, in_maps, core_ids=list(range(NCORES)))

    ctx = np.empty((B, E), dtype=np.float32)
    attw = np.empty((B, S), dtype=np.float32)
    for c in range(NCORES):
        r = res.results[c]
        exp_rows = r["expr"].astype(np.float32)          # (BL, S)
        z = r["zp"].astype(np.float32).sum(axis=1)       # (BL,)
        ctxc = r["ctxc"].astype(np.float32)              # (BL, P, KC)
        for b in range(BL):
            gb = c * BL + b
            attw[gb] = exp_rows[b] / z[b]
            # ctxc[b, p, k] -> ctx[e] with e = k*128 + p
            ctx[gb] = ctxc[b].T.reshape(E) / z[b]
    return ctx, attw


# revision 27
# speedup vs baseline: 1.2060x; 1.2047x over previous
"""Bahdanau additive-attention kernel for Trainium2, 8 NeuronCores. v3.

Problem (B=32, S=2048, H=1024, E=2H):
    hid_proj = hidden @ w_h.T + attn_b                  # (B, H)   host
    enc_proj[b,s,h] = sum_e enc[b,s,e] * w_e[h,e]       # (B, S, H) PE (dominant)
    energy = tanh(hid_proj[:,None,:] + enc_proj)        # ACT
    scores[b,s] = sum_h energy[b,s,h] * v_w[h]          # PE (v-dot)
    attw = softmax(scores, axis=1)                      # exp on ACT, /Z on host
    context[b,e] = sum_s attw[b,s] * enc[b,s,e]         # DVE (mult+reduce)

Sharding: data-parallel over batch, 4 batches per core.

Design notes (vs the 527us v1 baseline):
  - enc pre-transposed ON HOST to [b, e, s] bf16: plain contiguous DMAs
    instead of 8.3us xbar DMA-transposes; one resident tile serves both
    the main matmul (rhs) and the context reduction (in0). Halves HBM.
  - context off the PE (was 55us rank-1 matmuls + 64 transposes): exp row
    replicated across partitions via K=128 ones-matmul on a zero-padded
    row tile (K=1 replication returns zeros on HW), then per (st, k)
    DVE tensor_tensor mult + tensor_reduce partials, accumulated with a
    final tiny reduce. Per-st pipelining keeps all but the last s-tile's
    DVE work hidden under the next main matmuls. (Fused
    tensor_tensor_reduce hangs the device - do not use.)
  - softmax 1/Z on host: kernel outputs raw bf16 exp row, Z partials,
    unnormalized ctx columns.
  - startup: 16 PE warmup matmuls (HAM un-throttles after ~3.4us of
    activity); weights loaded m-major on the Scalar queue while batch 0's
    encT arrives in (k, st)-pieces on Sync/GpSimd, so the first m-loop
    starts at ~4us with no DMA stall (a stall >3.4us re-throttles the PE
    clock to 1.2GHz - costs ~45us).
  - MM issue rate is 216ns at N=512 with per-MM LDWEIGHTS (background
    weight-buffer load is free); no stationary-reuse tricks needed.
"""

import numpy as np
import ml_dtypes

import concourse.bass as bass
import concourse.tile as tile
import concourse.mybir as mybir
from concourse import bacc
from concourse.bass_utils import run_bass_kernel_spmd

B, S, H = 32, 2048, 1024
E = 2 * H
NCORES = 8
BL = B // NCORES          # batches per core
P = 128                   # partitions
KC = E // P               # 16 contraction chunks
MC = H // P               # 8 h chunks
NT = 512                  # moving free-dim per matmul (1 PSUM bank of fp32)
ST = S // NT              # 4 s-tiles per batch

F32 = mybir.dt.float32
BF16 = mybir.dt.bfloat16
BF16_NP = ml_dtypes.bfloat16


def build_nc():
    nc = bacc.Bacc("TRN2", target_bir_lowering=False, debug=False)

    # encT[b, k, p, s] = enc[b, s, k*128+p], pre-transposed + bf16 on host
    enc_in = nc.dram_tensor("encT", [BL, KC, P, S], BF16, kind="ExternalInput")
    # wT[k, p, h] = w_e[h, k*128+p]. Keep w_sb k-major [P, KC, H]: an
    # m-major layout measures 250ns/MM instead of 216 (SBUF read-path
    # conflict between the LDWEIGHTS and rhs streams).
    w_in = nc.dram_tensor("wT", [KC, P, H], BF16, kind="ExternalInput")
    v_in = nc.dram_tensor("vcol", [P, MC], BF16, kind="ExternalInput")
    hid_in = nc.dram_tensor("hidc", [P, BL, MC], F32, kind="ExternalInput")
    ctx_out = nc.dram_tensor("ctxc", [BL, P, KC], F32, kind="ExternalOutput")
    exp_out = nc.dram_tensor("expr", [BL, S], BF16, kind="ExternalOutput")
    z_out = nc.dram_tensor("zp", [BL, ST], F32, kind="ExternalOutput")

    with tile.TileContext(nc) as tc:
        with (
            tc.tile_pool(name="singles", bufs=1) as singles,
            tc.tile_pool(name="pencT", bufs=2) as pencT,
            tc.tile_pool(name="pen", bufs=2) as pen,
            tc.tile_pool(name="prow", bufs=1) as prow,
            tc.tile_pool(name="pmm", bufs=3, space="PSUM") as pmm,
            tc.tile_pool(name="psc", bufs=2, space="PSUM") as psc,
            tc.tile_pool(name="prep", bufs=2, space="PSUM") as prep,
        ):
            # --- warmup operands (memset only - ready in ~1us) -----------
            warm = singles.tile([P, P], BF16)
            nc.vector.memset(warm, 0.5)
            exrow_pad = singles.tile([P, S], BF16)
            nc.vector.memset(exrow_pad[:, 0:NT], 0.0)
            nc.vector.memset(exrow_pad[:, NT:S], 0.0)
            ones128 = singles.tile([P, P], BF16)
            nc.vector.memset(ones128, 1.0)

            # --- PE warmup: junk matmuls (~3.5us) to lift HAM to 2.4GHz
            for wi in range(16):
                wp = prep.tile([P, NT], F32, name=f"warm{wi}", tag="rep")
                nc.tensor.matmul(wp, lhsT=warm, rhs=exrow_pad[:, 0:NT],
                                 start=True, stop=True)

            # Scalar queue stays free for ACT (a busy Scalar queue delays
            # the first tanh and stalls the PE on PSUM-pool rotation)
            w_sb = singles.tile([P, KC, H], BF16)
            v_sb = singles.tile([P, MC], BF16)
            nc.gpsimd.dma_start(out=v_sb, in_=v_in[:, :])
            hid_sb = singles.tile([P, BL, MC], F32)
            nc.gpsimd.dma_start(out=hid_sb, in_=hid_in[:, :, :])

            exrep = singles.tile([P, S], BF16)
            scratch = singles.tile([P, NT], BF16)

            # --- startup streaming, paced to the first m-loop ------------
            # m0's k-loop needs w[k] + encT[st0, k] in k-order: interleave
            # them across Sync/GpSimd so chunk k lands just before its MM.
            encT = {}
            encT[0] = pencT.tile([P, KC, S], BF16, name="encT0", tag="encT")
            for k in range(KC):
                qa, qb = (nc.sync, nc.gpsimd) if k % 2 == 0 else (nc.gpsimd, nc.sync)
                qa.dma_start(out=w_sb[:, k, :], in_=w_in[k])
                qb.dma_start(out=encT[0][:, k, 0:NT], in_=enc_in[0, k, :, 0:NT])
            for st in range(1, ST):
                for k in range(KC):
                    q = nc.gpsimd if k % 2 else nc.sync
                    q.dma_start(out=encT[0][:, k, st * NT:(st + 1) * NT],
                                in_=enc_in[0, k, :, st * NT:(st + 1) * NT])
            for b in range(1, BL):
                encT[b] = pencT.tile([P, KC, S], BF16, name=f"encT{b}", tag="encT")
                nc.sync.dma_start(
                    out=encT[b][:, 0:KC // 2, :],
                    in_=enc_in[b, 0:KC // 2].rearrange("k p s -> p k s"))
                nc.gpsimd.dma_start(
                    out=encT[b][:, KC // 2:KC, :],
                    in_=enc_in[b, KC // 2:KC].rearrange("k p s -> p k s"))

            for b in range(BL):
                parts = prow.tile([P, KC, ST], F32, name=f"parts{b}", tag="parts")
                for st in range(ST):
                    # ---- main: enc_proj + tanh, one m-chunk at a time ---
                    en = pen.tile([P, MC, NT], BF16, name=f"en{b}_{st}", tag="en")
                    for m in range(MC):
                        ps = pmm.tile([P, NT], F32, name=f"ps{b}_{st}_{m}", tag="mm")
                        for k in range(KC):
                            nc.tensor.matmul(
                                ps,
                                lhsT=w_sb[:, k, m * P:(m + 1) * P],
                                rhs=encT[b][:, k, st * NT:(st + 1) * NT],
                                start=(k == 0),
                                stop=(k == KC - 1),
                            )
                        nc.scalar.activation(
                            out=en[:, m, :],
                            in_=ps,
                            func=mybir.ActivationFunctionType.Tanh,
                            bias=hid_sb[:, b, m:m + 1],
                            scale=1.0,
                        )
                    # ---- scores: v-dot on PE ----------------------------
                    sc = psc.tile([1, NT], F32, name=f"sc{b}_{st}", tag="sc")
                    for m in range(MC):
                        nc.tensor.matmul(
                            sc,
                            lhsT=v_sb[:, m:m + 1],
                            rhs=en[:, m, :],
                            start=(m == 0),
                            stop=(m == MC - 1),
                        )
                    # ---- exp straight from PSUM into padded row 0 -------
                    zt = prow.tile([1, 1], F32, name=f"z{b}_{st}", tag=f"z{st}")
                    nc.scalar.activation(
                        out=exrow_pad[0:1, st * NT:(st + 1) * NT],
                        in_=sc,
                        func=mybir.ActivationFunctionType.Exp,
                        accum_out=zt,
                    )
                    nc.scalar.dma_start(out=z_out[b, st:st + 1], in_=zt)

                    # ---- replicate exp slice across partitions ----------
                    rp = prep.tile([P, NT], F32, name=f"rp{b}_{st}", tag="rep")
                    nc.tensor.matmul(rp, lhsT=ones128,
                                     rhs=exrow_pad[:, st * NT:(st + 1) * NT],
                                     start=True, stop=True)
                    nc.scalar.activation(
                        out=exrep[:, st * NT:(st + 1) * NT],
                        in_=rp,
                        func=mybir.ActivationFunctionType.Copy,
                    )

                    # ---- context partials on DVE ------------------------
                    for k in range(KC):
                        nc.vector.tensor_tensor(
                            out=scratch,
                            in0=encT[b][:, k, st * NT:(st + 1) * NT],
                            in1=exrep[:, st * NT:(st + 1) * NT],
                            op=mybir.AluOpType.mult)
                        nc.vector.tensor_reduce(
                            out=parts[:, k, st:st + 1], in_=scratch,
                            axis=mybir.AxisListType.XYZW,
                            op=mybir.AluOpType.add)

                # ---- batch tail: raw outputs ----------------------------
                nc.scalar.dma_start(out=exp_out[b], in_=exrow_pad[0:1, :])
                ctxcols = prow.tile([P, KC], F32, name=f"cc{b}", tag="cc")
                nc.vector.tensor_reduce(out=ctxcols, in_=parts,
                                        axis=mybir.AxisListType.X,
                                        op=mybir.AluOpType.add)
                nc.gpsimd.dma_start(out=ctx_out[b], in_=ctxcols)

    nc.compile()
    return nc


_CACHE = {}


def _get_nc():
    if "nc" not in _CACHE:
        _CACHE["nc"] = build_nc()
    return _CACHE["nc"]


def prep_in_maps(hidden, encoder_outputs, attn_w, attn_b, v_w):
    hidden = np.asarray(hidden, dtype=np.float32)
    enc = np.asarray(encoder_outputs, dtype=np.float32)
    attn_w = np.asarray(attn_w, dtype=np.float32)
    attn_b = np.asarray(attn_b, dtype=np.float32)
    v_w = np.asarray(v_w, dtype=np.float32)

    w_h = attn_w[:, :H]                       # (H, H)
    w_e = attn_w[:, H:]                       # (H, E)
    hid_proj = hidden @ w_h.T + attn_b        # (B, H) fp32, exact
    wT = np.ascontiguousarray(w_e.T).astype(BF16_NP).reshape(KC, P, H)
    vcol = np.ascontiguousarray(v_w.reshape(MC, P).T).astype(BF16_NP)  # (P, MC)

    # pre-transpose enc to [b, e, s] bf16, viewed as [b, KC, P, S]
    encT = np.ascontiguousarray(
        enc.transpose(0, 2, 1).astype(BF16_NP)
    ).reshape(B, KC, P, S)

    in_maps = []
    for c in range(NCORES):
        hp = hid_proj[c * BL:(c + 1) * BL]    # (BL, H)
        hidc = np.ascontiguousarray(hp.reshape(BL, MC, P).transpose(2, 0, 1))
        in_maps.append({
            "encT": encT[c * BL:(c + 1) * BL],
            "wT": wT,
            "vcol": vcol,
            "hidc": hidc.astype(np.float32),
        })
    return in_maps


def kernel(hidden, encoder_outputs, attn_w, attn_b, v_w):
    in_maps = prep_in_maps(hidden, encoder_outputs, attn_w, attn_b, v_w)
    nc = _get_nc()
    res = run_bass_kernel_spmd(nc, in_maps, core_ids=list(range(NCORES)))

    ctx = np.empty((B, E), dtype=np.float32)
    attw = np.empty((B, S), dtype=np.float32)
    for c in range(NCORES):
        r = res.results[c]
        exp_rows = r["expr"].astype(np.float32)          # (BL, S)
        z = r["zp"].astype(np.float32).sum(axis=1)       # (BL,)
        ctxc = r["ctxc"].astype(np.float32)              # (BL, P, KC)
        for b in range(BL):
            gb = c * BL + b
            attw[gb] = exp_rows[b] / z[b]
            # ctxc[b, p, k] -> ctx[e] with e = k*128 + p
            ctx[gb] = ctxc[b].T.reshape(E) / z[b]
    return ctx, attw
